# revision 3
# baseline (speedup 1.0000x reference)
"""Position Attention Module (DANet) on 8 Trainium2 NeuronCores.

Reference computation (per batch b of 4):
  xf = x[b] : [C=512, N=4096]
  q = Wq@xf + bq : [64, N];  k = Wk@xf + bk : [64, N];  v = Wv@xf + bv : [512, N]
  scores[i,j] = q[:,i].k[:,j];  attn = softmax_j(scores)
  out[c,i] = alpha * sum_j v[c,j] attn[i,j]

Sharding: 2 cores per batch, each core owns half the query rows (i), full k/v.
Per-core x is pre-rolled on host so the owned i-half is always columns 0:2048.

v2 design (vs v1 f32r/fp32 baseline at 326us):
  - x streamed once (host f32r-rounded); q/k/v projections all single-f32r
    matmuls (1 cyc/row; fp32 was 4).  q/k host-scaled by 8 so their fp8
    splits stay clear of the e4m3 subnormal floor; exp scale compensates.
  - q/k projections emit duplicated rows ([k;k] via [WkT|WkT] weights) so
    every fp8 split/cast writes partition-aligned (no cross-partition ops).
  - scores: e4m3 3-way split (h/m/l), 2 DoubleRow fp8 matmuls per tile
    (8 K=64 slot products: hh,mh,hm,mm,lh,hl,ml,lm) at 0.5 cyc/row; exact
    to ~2^-12.  Predicted output rel-err ~2.5e-3 (numpy-sim validated).
  - scores issued 2 j-iterations ahead of the AV matmuls; exp latency hides
    under the AV group, removing the per-j PE stall seen in the v1 trace.
  - softmax denominator accumulated on the (otherwise idle) Pool engine;
    ones[128,128] f32r matmul broadcasts partition-sums; DVE reciprocal.
  - AV: out[c,i] += vT[j,c].T @ exp[j,i] in f32r (dense roofline term).
"""
import numpy as np


def _round_f32r(a):
    """Round fp32 mantissa to 11 bits (f32r / E8M11), round-half-even."""
    m, e = np.frexp(np.asarray(a, np.float32).astype(np.float64))
    return np.ldexp(np.round(np.ldexp(m, 12)), -12).astype(np.float32) * (2.0 ** e).astype(np.float32)


B, C, HW = 4, 512, 4096
CQ = 64
NCORES = 8
IH = HW // 2          # 2048 query rows per core
ITILE = 512           # i-tile (psum free dim)
NITILES = IH // ITILE # 4
JT = 128              # j-tile (contraction chunk for AV / scores lhsT cols)
NJT = HW // JT        # 32
JB = 512              # j-block for projections
NJB = HW // JB        # 8
NCC = C // 128        # 4 contraction chunks of 128 over C
QKS = 8.0             # host scale on q/k; exp rescales by 1/QKS^2
PIPE = 2              # scores run this many j-iterations ahead of AV

_cache = {}


def _build():
    import concourse.bacc as bacc
    import concourse.tile as tile
    import concourse.mybir as mybir
    from concourse.bass_utils import run_bass_kernel_spmd

    f32 = mybir.dt.float32
    f32r = mybir.dt.float32r
    e4 = mybir.dt.float8e4
    AF = mybir.ActivationFunctionType
    DR = mybir.MatmulPerfMode.DoubleRow

    nc = bacc.Bacc("TRN2", target_bir_lowering=False, debug=False)

    x_d = nc.dram_tensor("x", [C, HW], f32, kind="ExternalInput")
    wkk_d = nc.dram_tensor("wkk", [C, 128], f32, kind="ExternalInput")
    wqq_d = nc.dram_tensor("wqq", [C, 128], f32, kind="ExternalInput")
    wvt_d = nc.dram_tensor("wvt", [C, C], f32, kind="ExternalInput")
    bkk_d = nc.dram_tensor("bkk", [128, 1], f32, kind="ExternalInput")
    bqq_d = nc.dram_tensor("bqq", [128, 1], f32, kind="ExternalInput")
    bv_d = nc.dram_tensor("bv", [1, C], f32, kind="ExternalInput")
    out_d = nc.dram_tensor("out", [C, IH], f32, kind="ExternalOutput")

    with tile.TileContext(nc) as tc:
        with (
            tc.tile_pool(name="const", bufs=1) as cpool,
            tc.tile_pool(name="vt", bufs=1) as vtpool,
        ):
            # --- constants / weights ---
            wkk = [cpool.tile([128, 128], f32r, tag=f"wkk{i}", name=f"wkk{i}") for i in range(NCC)]
            wqq = [cpool.tile([128, 128], f32r, tag=f"wqq{i}", name=f"wqq{i}") for i in range(NCC)]
            wvt = [cpool.tile([128, C], f32r, tag=f"wvt{i}", name=f"wvt{i}") for i in range(NCC)]
            for cc in range(NCC):
                sl = slice(cc * 128, (cc + 1) * 128)
                nc.sync.dma_start(wkk[cc][:], wkk_d[sl, :].bitcast(f32r))
                nc.sync.dma_start(wqq[cc][:], wqq_d[sl, :].bitcast(f32r))
                nc.sync.dma_start(wvt[cc][:], wvt_d[sl, :].bitcast(f32r))
            bkk_c = cpool.tile([128, 1], f32, tag="bkkc")
            bqq_c = cpool.tile([128, 1], f32, tag="bqqc")
            nc.sync.dma_start(bkk_c[:], bkk_d[:])
            nc.sync.dma_start(bqq_c[:], bqq_d[:])
            bv_row = cpool.tile([1, C], f32, tag="bvrow")
            nc.sync.dma_start(bv_row[:], bv_d[:])
            ones_r = cpool.tile([1, 128], f32, tag="onesr")      # K=1 bcast lhsT
            nc.vector.memset(ones_r[:], 1.0)
            ones_sq = cpool.tile([128, 128], f32r, tag="onessq")  # sum+bcast lhsT
            nc.vector.memset(ones_sq[:].bitcast(f32), 1.0)

            # fp8 stacked score operands.  DoubleRow matmul computes
            # w[:,0].T@if[:,0] + w[:,1].T@if[:,1]; each w/if column stacks two
            # 64-row components, giving 4 K=64 slot products per matmul:
            #   A: [kh;km]x[qh;qh] + [kh;km]x[qm;qm] -> hh, mh, hm, mm
            #   B: [kl;kh]x[qh;ql] + [km;kl]x[ql;qm] -> lh, hl, ml, lm
            KA = cpool.tile([128, 2, HW], e4, tag="KA")
            KB = cpool.tile([128, 2, HW], e4, tag="KB")
            QA = cpool.tile([128, 2, IH], e4, tag="QA")
            QB = cpool.tile([128, 2, IH], e4, tag="QB")
            vts = [vtpool.tile([JT, C], f32r, tag=f"vt{j}", name=f"vt{j}") for j in range(NJT)]

            # bvB: (alpha*bv) broadcast to 128 partitions (for vT psum eviction)
            with tc.tile_pool(name="ppre", bufs=1, space="PSUM") as ppre:
                bvB = cpool.tile([128, C], f32, tag="bvB")
                ps = ppre.tile([128, C], f32, tag="bvps")
                nc.tensor.matmul(ps[:], ones_r[:], bv_row[:], start=True, stop=True)
                nc.vector.tensor_copy(bvB[:], ps[:])

            # ---------------- projections + fp8 splits ----------------
            with (
                tc.tile_pool(name="xin", bufs=8) as xpool,
                tc.tile_pool(name="evt", bufs=3) as evpool,
                tc.tile_pool(name="res", bufs=4) as rspool,
                tc.tile_pool(name="pkq", bufs=2, space="PSUM") as pkq,
                tc.tile_pool(name="pvt", bufs=3, space="PSUM") as pvt,
            ):
                for jb in range(NJB):
                    jsl = slice(jb * JB, (jb + 1) * JB)
                    xt = []
                    for cc in range(NCC):
                        csl = slice(cc * 128, (cc + 1) * 128)
                        t = xpool.tile([128, JB], f32r, tag="x", name=f"x{jb}_{cc}")
                        nc.sync.dma_start(t[:], x_d[csl, jsl].bitcast(f32r))
                        xt.append(t)
                    # k, duplicated on both halves: [k; k] = [WkT|WkT].T @ x
                    kp = pkq.tile([128, JB], f32, tag="kqp")
                    for cc in range(NCC):
                        nc.tensor.matmul(kp[:], wkk[cc][:], xt[cc][:],
                                         start=(cc == 0), stop=(cc == NCC - 1))
                    kf = evpool.tile([128, JB], f32, tag="ev")
                    nc.scalar.activation(kf[:], kp[:], AF.Identity, bias=bkk_c[:])
                    # 3-way e4m3 split; all casts/subs partition-aligned
                    nc.scalar.activation(KA[:, 0, jsl], kf[:], AF.Copy)          # [kh; kh]
                    r1 = rspool.tile([128, JB], f32, tag="r1")
                    nc.vector.tensor_sub(r1[:], kf[:], KA[:, 0, jsl])            # [r1; r1]
                    nc.scalar.activation(KA[64:128, 0, jsl], r1[64:128, :], AF.Copy)   # km (lower)
                    nc.scalar.activation(KB[0:64, 1, jsl], r1[0:64, :], AF.Copy)       # km (upper)
                    nc.scalar.activation(KB[64:128, 0, jsl], kf[64:128, :], AF.Copy)   # kh (lower)
                    r2 = rspool.tile([128, JB], f32, tag="r2")
                    nc.vector.tensor_sub(r2[0:64, :], r1[0:64, :], KB[0:64, 1, jsl])
                    nc.vector.tensor_sub(r2[64:128, :], r1[64:128, :], KA[64:128, 0, jsl])
                    nc.scalar.activation(KB[0:64, 0, jsl], r2[0:64, :], AF.Copy)       # kl (upper)
                    nc.scalar.activation(KB[64:128, 1, jsl], r2[64:128, :], AF.Copy)   # kl (lower)
                    nc.sync.dma_start(KA[:, 1, jsl], KA[:, 0, jsl])                    # [kh; km] dup
                    if jb < NJB // 2:
                        qp = pkq.tile([128, JB], f32, tag="kqp")
                        for cc in range(NCC):
                            nc.tensor.matmul(qp[:], wqq[cc][:], xt[cc][:],
                                             start=(cc == 0), stop=(cc == NCC - 1))
                        qf = evpool.tile([128, JB], f32, tag="ev")
                        nc.scalar.activation(qf[:], qp[:], AF.Identity, bias=bqq_c[:])
                        nc.scalar.activation(QA[:, 0, jsl], qf[:], AF.Copy)            # [qh; qh]
                        r1q = rspool.tile([128, JB], f32, tag="r1")
                        nc.vector.tensor_sub(r1q[:], qf[:], QA[:, 0, jsl])
                        nc.scalar.activation(QA[:, 1, jsl], r1q[:], AF.Copy)           # [qm; qm]
                        r2q = rspool.tile([128, JB], f32, tag="r2")
                        nc.vector.tensor_sub(r2q[:], r1q[:], QA[:, 1, jsl])
                        nc.scalar.activation(QB[0:64, 0, jsl], qf[0:64, :], AF.Copy)   # qh (upper)
                        nc.scalar.activation(QB[64:128, 0, jsl], r2q[64:128, :], AF.Copy)  # ql (lower)
                        nc.scalar.activation(QB[0:64, 1, jsl], r2q[0:64, :], AF.Copy)      # ql (upper)
                        nc.scalar.activation(QB[64:128, 1, jsl], r1q[64:128, :], AF.Copy)  # qm (lower)
                    # vT tiles [128 j, C] in f32r
                    for js in range(JB // JT):
                        vp = pvt.tile([JT, C], f32, tag="vtp")
                        for cc in range(NCC):
                            nc.tensor.matmul(
                                vp[:], xt[cc][:, js * JT:(js + 1) * JT], wvt[cc][:],
                                start=(cc == 0), stop=(cc == NCC - 1))
                        nc.vector.tensor_add(vts[jb * 4 + js][:], vp[:], bvB[:])

            # ---------------- attention ----------------
            with (
                tc.tile_pool(name="expp", bufs=4) as epool,
                tc.tile_pool(name="dnm", bufs=2) as dpool,
                tc.tile_pool(name="ost", bufs=4) as opool,
                tc.tile_pool(name="rows", bufs=2) as rpool,
                tc.tile_pool(name="pso", bufs=3, space="PSUM") as pso,
                tc.tile_pool(name="pout", bufs=5, space="PSUM") as pout,
            ):
                for it in range(NITILES):
                    isl = slice(it * ITILE, (it + 1) * ITILE)
                    ops = [pout.tile([128, ITILE], f32, tag="op", name=f"op{it}_{i}") for i in range(NCC)]
                    dnm = dpool.tile([128, ITILE], f32r, tag="dn")
                    ets = {}
                    for step in range(NJT + PIPE):
                        if step < NJT:
                            j = step
                            jsl = slice(j * JT, (j + 1) * JT)
                            sp = pso.tile([JT, ITILE], f32, tag="sc")
                            nc.tensor.matmul(sp[:], KA[:, :, jsl], QA[:, :, isl],
                                             start=True, stop=False, perf_mode=DR)
                            nc.tensor.matmul(sp[:], KB[:, :, jsl], QB[:, :, isl],
                                             start=False, stop=True, perf_mode=DR)
                            et = epool.tile([JT, ITILE], f32r, tag="exp")
                            nc.scalar.activation(et[:], sp[:], AF.Exp, scale=1.0 / (QKS * QKS))
                            ets[j] = et
                        if step >= PIPE:
                            jd = step - PIPE
                            et = ets.pop(jd)
                            if jd == 0:
                                nc.gpsimd.tensor_copy(dnm[:], et[:])
                            else:
                                nc.gpsimd.tensor_add(dnm[:], dnm[:], et[:])
                            for cc in range(NCC):
                                nc.tensor.matmul(
                                    ops[cc][:], vts[jd][:, cc * 128:(cc + 1) * 128], et[:],
                                    start=(jd == 0), stop=(jd == NJT - 1))
                    # denomB = column-sums of dnm broadcast to all 128 partitions
                    dB = pso.tile([128, ITILE], f32, tag="sc")
                    nc.tensor.matmul(dB[:], ones_sq[:], dnm[:], start=True, stop=True)
                    recipB = rpool.tile([128, ITILE], f32, tag="recipB")
                    nc.vector.reciprocal_approx_fast(out=recipB[:], in_=dB[:])
                    for cc in range(NCC):
                        ot = opool.tile([128, ITILE], f32, tag="ot")
                        nc.vector.tensor_mul(ot[:], ops[cc][:], recipB[:])
                        nc.sync.dma_start(out_d[cc * 128:(cc + 1) * 128, isl], ot[:])

    nc.compile()
    return nc, run_bass_kernel_spmd


def _host_inputs(x, Wq, bq, Wk, bk, Wv, bv, alpha):
    x = np.ascontiguousarray(np.asarray(x, dtype=np.float32)).reshape(B, C, HW)
    a = float(np.asarray(alpha, np.float32).reshape(-1)[0])
    wkkt = np.asarray(Wk, np.float32).T * QKS
    wkk = _round_f32r(np.ascontiguousarray(np.concatenate([wkkt, wkkt], axis=1)))
    wqqt = np.asarray(Wq, np.float32).T * QKS
    wqq = _round_f32r(np.ascontiguousarray(np.concatenate([wqqt, wqqt], axis=1)))
    wvt = _round_f32r(np.ascontiguousarray(np.asarray(Wv, np.float32).T * a))
    bkh = np.asarray(bk, np.float32).reshape(CQ) * QKS
    bkk = np.ascontiguousarray(np.concatenate([bkh, bkh]).reshape(128, 1))
    bqh = np.asarray(bq, np.float32).reshape(CQ) * QKS
    bqq = np.ascontiguousarray(np.concatenate([bqh, bqh]).reshape(128, 1))
    bva = (np.asarray(bv, np.float32) * a).reshape(1, C)

    in_maps = []
    for core in range(NCORES):
        b, ih = core // 2, core % 2
        xb = x[b]
        if ih:
            xb = np.concatenate([xb[:, IH:], xb[:, :IH]], axis=1)
        in_maps.append({"x": _round_f32r(np.ascontiguousarray(xb)),
                        "wkk": wkk, "wqq": wqq, "wvt": wvt,
                        "bkk": bkk, "bqq": bqq, "bv": bva})
    return in_maps


def kernel(x, Wq, bq, Wk, bk, Wv, bv, alpha, trace=False, trace_kwargs=None):
    if "nc" not in _cache:
        _cache["nc"] = _build()
    nc, run_spmd = _cache["nc"]

    in_maps = _host_inputs(x, Wq, bq, Wk, bk, Wv, bv, alpha)

    kwargs = {}
    if trace:
        kwargs["trace"] = True
        kwargs.update(trace_kwargs or {})
    res = run_spmd(nc, in_maps, list(range(NCORES)), **kwargs)

    out = np.empty((B, C, HW), dtype=np.float32)
    for core in range(NCORES):
        b, ih = core // 2, core % 2
        out[b][:, ih * IH:(ih + 1) * IH] = res.results[core]["out"]
    if trace:
        return out.reshape(B, C, 64, 64), res
    return out.reshape(B, C, 64, 64)


# revision 4
# speedup vs baseline: 1.2464x; 1.2464x over previous
"""Position Attention Module (DANet) on 8 Trainium2 NeuronCores.

Reference computation (per batch b of 4):
  xf = x[b] : [C=512, N=4096]
  q = Wq@xf + bq : [64, N];  k = Wk@xf + bk : [64, N];  v = Wv@xf + bv : [512, N]
  scores[i,j] = q[:,i].k[:,j];  attn = softmax_j(scores)
  out[c,i] = alpha * sum_j v[c,j] attn[i,j]

Sharding: 2 cores per batch, each core owns half the query rows (i), full k/v.
Per-core x is pre-rolled on host so the owned i-half is always columns 0:2048.

v3 design (v1 was 326us; v2's fp8 DoubleRow scores broke the PE's
weight-load/execute overlap and gained nothing):
  - x streamed once (host f32r-rounded); q/k/v projections all single-f32r
    matmuls (1 cyc/row; v1 used fp32 q/k at 4 cyc/row).  Costs ~2e-3 output
    error (numpy-sim validated) against a 2e-2 budget.
  - scoresT [j, i] via the v1 K-stacked f32r hi/lo decomposition: KHL =
    [k_hi; k_lo], scores = KHL.[q_hi;q_hi] + KHL.[q_lo;q_lo] = k.q.
  - scores issued PIPE j-iterations ahead of the AV matmuls so the exp (Act)
    latency hides under the AV group -- v1 stalled ~620ns per j-iteration
    waiting for exp through a 2-deep PSUM rotation.
  - softmax denominator accumulated on the (otherwise idle) Pool engine;
    ones[128,128] f32r matmul broadcasts partition-sums; DVE reciprocal.
  - AV: out[c,i] += vT[j,c].T @ exp[j,i] in f32r (dense roofline term).
"""
import numpy as np


def _round_f32r(a):
    """Round fp32 mantissa to 11 bits (f32r / E8M11), round-half-even."""
    m, e = np.frexp(np.asarray(a, np.float32).astype(np.float64))
    return np.ldexp(np.round(np.ldexp(m, 12)), -12).astype(np.float32) * (2.0 ** e).astype(np.float32)


B, C, HW = 4, 512, 4096
CQ = 64
NCORES = 8
IH = HW // 2          # 2048 query rows per core
ITILE = 512           # i-tile (psum free dim)
NITILES = IH // ITILE # 4
JT = 128              # j-tile (contraction chunk for AV / scores lhsT cols)
NJT = HW // JT        # 32
JB = 512              # j-block for projections
NJB = HW // JB        # 8
NCC = C // 128        # 4 contraction chunks of 128 over C
PIPE = 2              # scores run this many j-iterations ahead of AV

_cache = {}


def _build():
    import concourse.bacc as bacc
    import concourse.tile as tile
    import concourse.mybir as mybir
    from concourse.bass_utils import run_bass_kernel_spmd

    f32 = mybir.dt.float32
    f32r = mybir.dt.float32r
    AF = mybir.ActivationFunctionType

    nc = bacc.Bacc("TRN2", target_bir_lowering=False, debug=False)

    x_d = nc.dram_tensor("x", [C, HW], f32, kind="ExternalInput")
    wqt_d = nc.dram_tensor("wqt", [C, CQ], f32, kind="ExternalInput")
    wkt_d = nc.dram_tensor("wkt", [C, CQ], f32, kind="ExternalInput")
    wvt_d = nc.dram_tensor("wvt", [C, C], f32, kind="ExternalInput")
    bq_d = nc.dram_tensor("bq", [CQ, 1], f32, kind="ExternalInput")
    bk_d = nc.dram_tensor("bk", [CQ, 1], f32, kind="ExternalInput")
    bv_d = nc.dram_tensor("bv", [1, C], f32, kind="ExternalInput")
    out_d = nc.dram_tensor("out", [C, IH], f32, kind="ExternalOutput")

    with tile.TileContext(nc) as tc:
        with (
            tc.tile_pool(name="const", bufs=1) as cpool,
            tc.tile_pool(name="kq", bufs=1) as kqpool,
            tc.tile_pool(name="vt", bufs=1) as vtpool,
        ):
            # --- constants / weights ---
            wqt = [cpool.tile([128, CQ], f32r, tag=f"wqt{i}", name=f"wqt{i}") for i in range(NCC)]
            wkt = [cpool.tile([128, CQ], f32r, tag=f"wkt{i}", name=f"wkt{i}") for i in range(NCC)]
            wvt = [cpool.tile([128, C], f32r, tag=f"wvt{i}", name=f"wvt{i}") for i in range(NCC)]
            for cc in range(NCC):
                sl = slice(cc * 128, (cc + 1) * 128)
                nc.sync.dma_start(wqt[cc][:], wqt_d[sl, :].bitcast(f32r))
                nc.sync.dma_start(wkt[cc][:], wkt_d[sl, :].bitcast(f32r))
                nc.sync.dma_start(wvt[cc][:], wvt_d[sl, :].bitcast(f32r))
            bq_c = cpool.tile([CQ, 1], f32, tag="bqc")
            bk_c = cpool.tile([CQ, 1], f32, tag="bkc")
            nc.sync.dma_start(bq_c[:], bq_d[:])
            nc.sync.dma_start(bk_c[:], bk_d[:])
            bv_row = cpool.tile([1, C], f32, tag="bvrow")
            nc.sync.dma_start(bv_row[:], bv_d[:])
            ones_r = cpool.tile([1, 128], f32, tag="onesr")      # K=1 bcast lhsT
            nc.vector.memset(ones_r[:], 1.0)
            ones_sq = cpool.tile([128, 128], f32r, tag="onessq")  # sum+bcast lhsT
            nc.vector.memset(ones_sq[:].bitcast(f32), 1.0)

            # f32r hi/lo split activations for scores (K-stacked):
            #  KHL [128, HW]: rows 0-63 = k_hi, rows 64-127 = k_lo
            #  QHH [128, IH]: q_hi duplicated on both halves; QLL: q_lo dup
            KHL = kqpool.tile([128, HW], f32r, tag="khl")
            QHH = kqpool.tile([128, IH], f32r, tag="qhh")
            QLL = kqpool.tile([128, IH], f32r, tag="qll")
            vts = [vtpool.tile([JT, C], f32r, tag=f"vt{j}", name=f"vt{j}") for j in range(NJT)]

            # bvB: (alpha*bv) broadcast to 128 partitions (for vT psum eviction)
            with tc.tile_pool(name="ppre", bufs=1, space="PSUM") as ppre:
                bvB = cpool.tile([128, C], f32, tag="bvB")
                ps = ppre.tile([128, C], f32, tag="bvps")
                nc.tensor.matmul(ps[:], ones_r[:], bv_row[:], start=True, stop=True)
                nc.vector.tensor_copy(bvB[:], ps[:])

            # ---------------- projections ----------------
            with (
                tc.tile_pool(name="xin", bufs=8) as xpool,
                tc.tile_pool(name="evt", bufs=3) as evpool,
                tc.tile_pool(name="pkq", bufs=2, space="PSUM") as pkq,
                tc.tile_pool(name="pvt", bufs=3, space="PSUM") as pvt,
            ):
                for jb in range(NJB):
                    jsl = slice(jb * JB, (jb + 1) * JB)
                    xt = []
                    for cc in range(NCC):
                        csl = slice(cc * 128, (cc + 1) * 128)
                        t = xpool.tile([128, JB], f32r, tag="x", name=f"x{jb}_{cc}")
                        nc.sync.dma_start(t[:], x_d[csl, jsl].bitcast(f32r))
                        xt.append(t)
                    # k projection [64, JB] in f32r
                    kp = pkq.tile([CQ, JB], f32, tag="kqp")
                    for cc in range(NCC):
                        nc.tensor.matmul(kp[:], wkt[cc][:], xt[cc][:],
                                         start=(cc == 0), stop=(cc == NCC - 1))
                    ktmp = evpool.tile([CQ, JB], f32, tag="ev")
                    nc.scalar.activation(ktmp[:], kp[:], AF.Identity, bias=bk_c[:])
                    nc.vector.tensor_copy(KHL[0:CQ, jsl], ktmp[:])
                    klo = evpool.tile([CQ, JB], f32r, tag="evlo")
                    nc.vector.tensor_sub(klo[:], ktmp[:], KHL[0:CQ, jsl])
                    nc.sync.dma_start(KHL[CQ:128, jsl], klo[:])
                    if jb < NJB // 2:
                        qp = pkq.tile([CQ, JB], f32, tag="kqp")
                        for cc in range(NCC):
                            nc.tensor.matmul(qp[:], wqt[cc][:], xt[cc][:],
                                             start=(cc == 0), stop=(cc == NCC - 1))
                        qtmp = evpool.tile([CQ, JB], f32, tag="ev")
                        nc.scalar.activation(qtmp[:], qp[:], AF.Identity, bias=bq_c[:])
                        nc.vector.tensor_copy(QHH[0:CQ, jsl], qtmp[:])
                        nc.sync.dma_start(QHH[CQ:128, jsl], QHH[0:CQ, jsl])
                        nc.vector.tensor_sub(QLL[0:CQ, jsl], qtmp[:], QHH[0:CQ, jsl])
                        nc.sync.dma_start(QLL[CQ:128, jsl], QLL[0:CQ, jsl])
                    # vT tiles [128 j, C] in f32r
                    for js in range(JB // JT):
                        vp = pvt.tile([JT, C], f32, tag="vtp")
                        for cc in range(NCC):
                            nc.tensor.matmul(
                                vp[:], xt[cc][:, js * JT:(js + 1) * JT], wvt[cc][:],
                                start=(cc == 0), stop=(cc == NCC - 1))
                        nc.vector.tensor_add(vts[jb * 4 + js][:], vp[:], bvB[:])

            # ---------------- attention ----------------
            with (
                tc.tile_pool(name="expp", bufs=4) as epool,
                tc.tile_pool(name="dnm", bufs=2) as dpool,
                tc.tile_pool(name="ost", bufs=4) as opool,
                tc.tile_pool(name="rows", bufs=2) as rpool,
                tc.tile_pool(name="pso", bufs=3, space="PSUM") as pso,
                tc.tile_pool(name="pout", bufs=5, space="PSUM") as pout,
            ):
                for it in range(NITILES):
                    isl = slice(it * ITILE, (it + 1) * ITILE)
                    ops = [pout.tile([128, ITILE], f32, tag="op", name=f"op{it}_{i}") for i in range(NCC)]
                    dnm = dpool.tile([128, ITILE], f32r, tag="dn")
                    ets = {}
                    for step in range(NJT + PIPE):
                        if step < NJT:
                            j = step
                            jsl = slice(j * JT, (j + 1) * JT)
                            sp = pso.tile([JT, ITILE], f32, tag="sc")
                            nc.tensor.matmul(sp[:], KHL[:, jsl], QHH[:, isl],
                                             start=True, stop=False)
                            nc.tensor.matmul(sp[:], KHL[:, jsl], QLL[:, isl],
                                             start=False, stop=True)
                            et = epool.tile([JT, ITILE], f32r, tag="exp")
                            nc.scalar.activation(et[:], sp[:], AF.Exp)
                            ets[j] = et
                        if step >= PIPE:
                            jd = step - PIPE
                            et = ets.pop(jd)
                            if jd == 0:
                                nc.gpsimd.tensor_copy(dnm[:], et[:])
                            else:
                                nc.gpsimd.tensor_add(dnm[:], dnm[:], et[:])
                            for cc in range(NCC):
                                nc.tensor.matmul(
                                    ops[cc][:], vts[jd][:, cc * 128:(cc + 1) * 128], et[:],
                                    start=(jd == 0), stop=(jd == NJT - 1))
                    # denomB = column-sums of dnm broadcast to all 128 partitions
                    dB = pso.tile([128, ITILE], f32, tag="sc")
                    nc.tensor.matmul(dB[:], ones_sq[:], dnm[:], start=True, stop=True)
                    recipB = rpool.tile([128, ITILE], f32, tag="recipB")
                    nc.vector.reciprocal_approx_fast(out=recipB[:], in_=dB[:])
                    for cc in range(NCC):
                        ot = opool.tile([128, ITILE], f32, tag="ot")
                        nc.vector.tensor_mul(ot[:], ops[cc][:], recipB[:])
                        nc.sync.dma_start(out_d[cc * 128:(cc + 1) * 128, isl], ot[:])

    nc.compile()
    return nc, run_bass_kernel_spmd


def _host_inputs(x, Wq, bq, Wk, bk, Wv, bv, alpha):
    x = np.ascontiguousarray(np.asarray(x, dtype=np.float32)).reshape(B, C, HW)
    a = float(np.asarray(alpha, np.float32).reshape(-1)[0])
    wqt = _round_f32r(np.ascontiguousarray(np.asarray(Wq, np.float32).T))
    wkt = _round_f32r(np.ascontiguousarray(np.asarray(Wk, np.float32).T))
    wvt = _round_f32r(np.ascontiguousarray(np.asarray(Wv, np.float32).T * a))
    bq = np.asarray(bq, np.float32).reshape(CQ, 1)
    bk = np.asarray(bk, np.float32).reshape(CQ, 1)
    bva = (np.asarray(bv, np.float32) * a).reshape(1, C)

    in_maps = []
    for core in range(NCORES):
        b, ih = core // 2, core % 2
        xb = x[b]
        if ih:
            xb = np.concatenate([xb[:, IH:], xb[:, :IH]], axis=1)
        in_maps.append({"x": _round_f32r(np.ascontiguousarray(xb)),
                        "wqt": wqt, "wkt": wkt, "wvt": wvt,
                        "bq": bq, "bk": bk, "bv": bva})
    return in_maps


def kernel(x, Wq, bq, Wk, bk, Wv, bv, alpha, trace=False, trace_kwargs=None):
    if "nc" not in _cache:
        _cache["nc"] = _build()
    nc, run_spmd = _cache["nc"]

    in_maps = _host_inputs(x, Wq, bq, Wk, bk, Wv, bv, alpha)

    kwargs = {}
    if trace:
        kwargs["trace"] = True
        kwargs.update(trace_kwargs or {})
    res = run_spmd(nc, in_maps, list(range(NCORES)), **kwargs)

    out = np.empty((B, C, HW), dtype=np.float32)
    for core in range(NCORES):
        b, ih = core // 2, core % 2
        out[b][:, ih * IH:(ih + 1) * IH] = res.results[core]["out"]
    if trace:
        return out.reshape(B, C, 64, 64), res
    return out.reshape(B, C, 64, 64)


# revision 10
# speedup vs baseline: 1.4796x; 1.1871x over previous
"""Position Attention Module (DANet) on 8 Trainium2 NeuronCores.

Reference computation (per batch b of 4):
  xf = x[b] : [C=512, N=4096]
  q = Wq@xf + bq : [64, N];  k = Wk@xf + bk : [64, N];  v = Wv@xf + bv : [512, N]
  scores[i,j] = q[:,i].k[:,j];  attn = softmax_j(scores)
  out[c,i] = alpha * sum_j v[c,j] attn[i,j]

Sharding: 2 cores per batch, each core owns half the query rows (i), full k/v.
Per-core x is pre-rolled on host so the owned i-half is always columns 0:2048.

v3 design (v1 was 326us; v2's fp8 DoubleRow scores broke the PE's
weight-load/execute overlap and gained nothing):
  - x streamed once (host f32r-rounded); q/k/v projections all single-f32r
    matmuls (1 cyc/row; v1 used fp32 q/k at 4 cyc/row).  Costs ~2e-3 output
    error (numpy-sim validated) against a 2e-2 budget.
  - scoresT [j, i] in ONE f32r matmul per tile: KHL = [k_hi; k_lo] K-stack
    keeps k exact; q participates as q_hi only (f32r, 2^-12) -- the dropped
    k.q_lo term costs ~2e-3 score error, validated at 2.0e-3 output rel-err.
  - scores issued PIPE j-iterations ahead of the AV matmuls so the exp (Act)
    latency hides under the AV group -- v1 stalled ~620ns per j-iteration
    waiting for exp through a 2-deep PSUM rotation.
  - softmax denominator split into two accumulation chains (even j-tiles on
    the Pool engine, odd on DVE) to halve the serial-latency of the chain;
    a pair of ones[128,128] f32r matmuls sums+broadcasts; DVE reciprocal.
  - AV: out[c,i] += vT[j,c].T @ exp[j,i] in f32r (dense roofline term).
  - x jb0/jb1 tile DMAs issued before the wvt/bvB constants so the first
    projection matmul is not queued behind cold-start bulk transfers.
"""
import numpy as np


def _round_f32r(a):
    """Round fp32 mantissa to 11 bits (f32r / E8M11), round-half-even."""
    m, e = np.frexp(np.asarray(a, np.float32).astype(np.float64))
    return np.ldexp(np.round(np.ldexp(m, 12)), -12).astype(np.float32) * (2.0 ** e).astype(np.float32)


B, C, HW = 4, 512, 4096
CQ = 64
NCORES = 8
IH = HW // 2          # 2048 query rows per core
ITILE = 512           # i-tile (psum free dim)
NITILES = IH // ITILE # 4
JT = 128              # j-tile (contraction chunk for AV / scores lhsT cols)
NJT = HW // JT        # 32
JB = 512              # j-block for projections
NJB = HW // JB        # 8
NCC = C // 128        # 4 contraction chunks of 128 over C
PIPE = 2              # scores run this many j-iterations ahead of AV

_cache = {}


def _build():
    import concourse.bacc as bacc
    import concourse.tile as tile
    import concourse.mybir as mybir
    from concourse.bass_utils import run_bass_kernel_spmd

    f32 = mybir.dt.float32
    f32r = mybir.dt.float32r
    AF = mybir.ActivationFunctionType

    nc = bacc.Bacc("TRN2", target_bir_lowering=False, debug=False)

    x_d = nc.dram_tensor("x", [C, HW], f32, kind="ExternalInput")
    wqt_d = nc.dram_tensor("wqt", [C, CQ], f32, kind="ExternalInput")
    wkt_d = nc.dram_tensor("wkt", [C, CQ], f32, kind="ExternalInput")
    wvt_d = nc.dram_tensor("wvt", [C, C], f32, kind="ExternalInput")
    bq_d = nc.dram_tensor("bq", [CQ, 1], f32, kind="ExternalInput")
    bk_d = nc.dram_tensor("bk", [CQ, 1], f32, kind="ExternalInput")
    bv_d = nc.dram_tensor("bv", [1, C], f32, kind="ExternalInput")
    out_d = nc.dram_tensor("out", [C, IH], f32, kind="ExternalOutput")

    with tile.TileContext(nc) as tc:
        with (
            tc.tile_pool(name="const", bufs=1) as cpool,
            tc.tile_pool(name="kq", bufs=1) as kqpool,
            tc.tile_pool(name="vt", bufs=1) as vtpool,
        ):
            # --- constants / weights ---
            wqt = [cpool.tile([128, CQ], f32r, tag=f"wqt{i}", name=f"wqt{i}") for i in range(NCC)]
            wkt = [cpool.tile([128, CQ], f32r, tag=f"wkt{i}", name=f"wkt{i}") for i in range(NCC)]
            wvt = [cpool.tile([128, C], f32r, tag=f"wvt{i}", name=f"wvt{i}") for i in range(NCC)]
            for cc in range(NCC):
                sl = slice(cc * 128, (cc + 1) * 128)
                nc.sync.dma_start(wkt[cc][:], wkt_d[sl, :].bitcast(f32r))
                nc.sync.dma_start(wqt[cc][:], wqt_d[sl, :].bitcast(f32r))
            bq_c = cpool.tile([CQ, 1], f32, tag="bqc")
            bk_c = cpool.tile([CQ, 1], f32, tag="bkc")
            nc.sync.dma_start(bq_c[:], bq_d[:])
            nc.sync.dma_start(bk_c[:], bk_d[:])
            # f32r hi/lo split activations for scores (K-stacked):
            #  KHL [128, HW]: rows 0-63 = k_hi, rows 64-127 = k_lo
            #  QHH [128, IH]: q_hi duplicated on both halves
            KHL = kqpool.tile([128, HW], f32r, tag="khl")
            QHH = kqpool.tile([128, IH], f32r, tag="qhh")
            vts = [vtpool.tile([JT, C], f32r, tag=f"vt{j}", name=f"vt{j}") for j in range(NJT)]

            # ---------------- projections ----------------
            with (
                tc.tile_pool(name="xin", bufs=12) as xpool,
                tc.tile_pool(name="evt", bufs=3) as evpool,
                tc.tile_pool(name="pkq", bufs=2, space="PSUM") as pkq,
                tc.tile_pool(name="pvt", bufs=3, space="PSUM") as pvt,
            ):
                # prefetch the first two jb x-blocks ahead of the bulk consts
                xts = {}
                for jb in range(2):
                    for cc in range(NCC):
                        csl = slice(cc * 128, (cc + 1) * 128)
                        jsl = slice(jb * JB, (jb + 1) * JB)
                        t = xpool.tile([128, JB], f32r, tag="x", name=f"x{jb}_{cc}")
                        nc.sync.dma_start(t[:], x_d[csl, jsl].bitcast(f32r))
                        xts[(jb, cc)] = t
                for cc in range(NCC):
                    sl = slice(cc * 128, (cc + 1) * 128)
                    nc.sync.dma_start(wvt[cc][:], wvt_d[sl, :].bitcast(f32r))
                bv_row = cpool.tile([1, C], f32, tag="bvrow")
                nc.sync.dma_start(bv_row[:], bv_d[:])
                ones_r = cpool.tile([1, 128], f32, tag="onesr")      # K=1 bcast lhsT
                nc.vector.memset(ones_r[:], 1.0)
                ones_sq = cpool.tile([128, 128], f32r, tag="onessq")  # sum+bcast lhsT
                nc.vector.memset(ones_sq[:].bitcast(f32), 1.0)

                # bvB: (alpha*bv) broadcast to 128 partitions (for vT psum evict)
                bvB = cpool.tile([128, C], f32, tag="bvB")
                ps = pvt.tile([128, C], f32, tag="bvps")
                nc.tensor.matmul(ps[:], ones_r[:], bv_row[:], start=True, stop=True)
                nc.vector.tensor_copy(bvB[:], ps[:])

                for jb in range(NJB):
                    jsl = slice(jb * JB, (jb + 1) * JB)
                    xt = []
                    for cc in range(NCC):
                        if (jb, cc) in xts:
                            xt.append(xts.pop((jb, cc)))
                            continue
                        csl = slice(cc * 128, (cc + 1) * 128)
                        t = xpool.tile([128, JB], f32r, tag="x", name=f"x{jb}_{cc}")
                        nc.sync.dma_start(t[:], x_d[csl, jsl].bitcast(f32r))
                        xt.append(t)
                    # k projection [64, JB] in f32r
                    kp = pkq.tile([CQ, JB], f32, tag="kqp")
                    for cc in range(NCC):
                        nc.tensor.matmul(kp[:], wkt[cc][:], xt[cc][:],
                                         start=(cc == 0), stop=(cc == NCC - 1))
                    ktmp = evpool.tile([CQ, JB], f32, tag="ev")
                    nc.scalar.activation(ktmp[:], kp[:], AF.Identity, bias=bk_c[:])
                    nc.vector.tensor_copy(KHL[0:CQ, jsl], ktmp[:])
                    klo = evpool.tile([CQ, JB], f32r, tag="evlo")
                    nc.vector.tensor_sub(klo[:], ktmp[:], KHL[0:CQ, jsl])
                    nc.sync.dma_start(KHL[CQ:128, jsl], klo[:])
                    if jb < NJB // 2:
                        qp = pkq.tile([CQ, JB], f32, tag="kqp")
                        for cc in range(NCC):
                            nc.tensor.matmul(qp[:], wqt[cc][:], xt[cc][:],
                                             start=(cc == 0), stop=(cc == NCC - 1))
                        qtmp = evpool.tile([CQ, JB], f32, tag="ev")
                        nc.scalar.activation(qtmp[:], qp[:], AF.Identity, bias=bq_c[:])
                        nc.vector.tensor_copy(QHH[0:CQ, jsl], qtmp[:])
                        nc.sync.dma_start(QHH[CQ:128, jsl], QHH[0:CQ, jsl])
                    # vT tiles [128 j, C] in f32r
                    for js in range(JB // JT):
                        vp = pvt.tile([JT, C], f32, tag="vtp")
                        for cc in range(NCC):
                            nc.tensor.matmul(
                                vp[:], xt[cc][:, js * JT:(js + 1) * JT], wvt[cc][:],
                                start=(cc == 0), stop=(cc == NCC - 1))
                        nc.vector.tensor_add(vts[jb * 4 + js][:], vp[:], bvB[:])

            # ---------------- attention ----------------
            with (
                tc.tile_pool(name="expp", bufs=4) as epool,
                tc.tile_pool(name="dnm", bufs=4) as dpool,
                tc.tile_pool(name="ost", bufs=4) as opool,
                tc.tile_pool(name="rows", bufs=2) as rpool,
                tc.tile_pool(name="pso", bufs=3, space="PSUM") as pso,
                tc.tile_pool(name="pout", bufs=5, space="PSUM") as pout,
            ):
                for it in range(NITILES):
                    isl = slice(it * ITILE, (it + 1) * ITILE)
                    ops = [pout.tile([128, ITILE], f32, tag="op", name=f"op{it}_{i}") for i in range(NCC)]
                    dnmP = dpool.tile([128, ITILE], f32r, tag="dnp")
                    dnmV = dpool.tile([128, ITILE], f32r, tag="dnv")
                    ets = {}
                    for step in range(NJT + PIPE):
                        if step < NJT:
                            j = step
                            jsl = slice(j * JT, (j + 1) * JT)
                            sp = pso.tile([JT, ITILE], f32, tag="sc")
                            nc.tensor.matmul(sp[:], KHL[:, jsl], QHH[:, isl],
                                             start=True, stop=True)
                            et = epool.tile([JT, ITILE], f32r, tag="exp")
                            nc.scalar.activation(et[:], sp[:], AF.Exp)
                            ets[j] = et
                        if step >= PIPE:
                            jd = step - PIPE
                            et = ets.pop(jd)
                            # denominator: even j-tiles chain on Pool, odd on DVE
                            eng, dnm = (nc.gpsimd, dnmP) if jd % 2 == 0 else (nc.vector, dnmV)
                            if jd < 2:
                                eng.tensor_copy(dnm[:], et[:])
                            else:
                                eng.tensor_add(dnm[:], dnm[:], et[:])
                            for cc in range(NCC):
                                nc.tensor.matmul(
                                    ops[cc][:], vts[jd][:, cc * 128:(cc + 1) * 128], et[:],
                                    start=(jd == 0), stop=(jd == NJT - 1))
                    # denomB = column-sums of dnmP+dnmV broadcast to all partitions
                    dB = pso.tile([128, ITILE], f32, tag="sc")
                    nc.tensor.matmul(dB[:], ones_sq[:], dnmP[:], start=True, stop=False)
                    nc.tensor.matmul(dB[:], ones_sq[:], dnmV[:], start=False, stop=True)
                    recipB = rpool.tile([128, ITILE], f32, tag="recipB")
                    nc.vector.reciprocal_approx_fast(out=recipB[:], in_=dB[:])
                    for cc in range(NCC):
                        ot = opool.tile([128, ITILE], f32, tag="ot")
                        nc.vector.tensor_mul(ot[:], ops[cc][:], recipB[:])
                        nc.sync.dma_start(out_d[cc * 128:(cc + 1) * 128, isl], ot[:])

    nc.compile()
    return nc, run_bass_kernel_spmd


def _host_inputs(x, Wq, bq, Wk, bk, Wv, bv, alpha):
    x = np.ascontiguousarray(np.asarray(x, dtype=np.float32)).reshape(B, C, HW)
    a = float(np.asarray(alpha, np.float32).reshape(-1)[0])
    wqt = _round_f32r(np.ascontiguousarray(np.asarray(Wq, np.float32).T))
    wkt = _round_f32r(np.ascontiguousarray(np.asarray(Wk, np.float32).T))
    wvt = _round_f32r(np.ascontiguousarray(np.asarray(Wv, np.float32).T * a))
    bq = np.asarray(bq, np.float32).reshape(CQ, 1)
    bk = np.asarray(bk, np.float32).reshape(CQ, 1)
    bva = (np.asarray(bv, np.float32) * a).reshape(1, C)

    in_maps = []
    for core in range(NCORES):
        b, ih = core // 2, core % 2
        xb = x[b]
        if ih:
            xb = np.concatenate([xb[:, IH:], xb[:, :IH]], axis=1)
        in_maps.append({"x": _round_f32r(np.ascontiguousarray(xb)),
                        "wqt": wqt, "wkt": wkt, "wvt": wvt,
                        "bq": bq, "bk": bk, "bv": bva})
    return in_maps


def kernel(x, Wq, bq, Wk, bk, Wv, bv, alpha, trace=False, trace_kwargs=None):
    if "nc" not in _cache:
        _cache["nc"] = _build()
    nc, run_spmd = _cache["nc"]

    in_maps = _host_inputs(x, Wq, bq, Wk, bk, Wv, bv, alpha)

    kwargs = {}
    if trace:
        kwargs["trace"] = True
        kwargs.update(trace_kwargs or {})
    res = run_spmd(nc, in_maps, list(range(NCORES)), **kwargs)

    out = np.empty((B, C, HW), dtype=np.float32)
    for core in range(NCORES):
        b, ih = core // 2, core % 2
        out[b][:, ih * IH:(ih + 1) * IH] = res.results[core]["out"]
    if trace:
        return out.reshape(B, C, 64, 64), res
    return out.reshape(B, C, 64, 64)
